# revision 1
# baseline (speedup 1.0000x reference)
"""Trainium2 Bass kernel for ChronosMOEFeedForward (8-expert top-2 MoE + shared expert).

All matmul data is bf16 (fp32 PSUM accumulation): 4x the PE rate of fp32r as
measured on HW and half the HBM traffic; weight/activation layouts are
pre-tiled host-side so every DMA is a contiguous per-partition block, weight
DMAs stream in progressive groups, x/weights/outputs are split across both
HWDGE rings to keep the first matmul's dependencies off the serial FIFO, and
the PSUM pool uses 6 of 8 banks (8 deadlocks on HW).

Strategy (expert-parallel over 8 NeuronCores):
  - Host computes the (tiny) gate: softmax(x @ gate_w.T), top-2, normalized
    combine weights; dispatches each token to its 2 experts.
  - Core e runs a SwiGLU FFN over the tokens routed to expert e (gathered,
    transposed, padded to capacity C), scaling rows by the combine weight
    during PSUM eviction. Each core also processes a 1024-token x I/4 slice
    of the shared expert.
  - Host scatter-adds routed outputs and concatenates shared slices (fp32).

Fixed problem shapes: x [2,1024,1024], E=8 experts, H=1024, I=2048, top-2,
one shared expert. The device program is compiled per capacity C (multiple
of 128 covering the max per-expert token count) and cached in-process.
"""
import math
from contextlib import ExitStack

import numpy as np
import ml_dtypes

import concourse.bass as bass
import concourse.tile as tile
from concourse import bacc, mybir
from concourse.bass_utils import run_bass_kernel_spmd

F32 = mybir.dt.float32
BF16 = mybir.dt.bfloat16
ActFn = mybir.ActivationFunctionType
BF = ml_dtypes.bfloat16

P = 128
B, S, H = 2, 1024, 1024
T = B * S                    # 2048 tokens
E, TOPK, I = 8, 2, 2048
NCORES = 8
TG = 2                       # shared expert: token groups
SI = 4                       # shared expert: I-dimension split
CS = T // TG                 # shared-expert tokens per core (1024)
SIC = I // P // SI           # shared-expert I-chunks per core (4)
HC = H // P                  # 8 H-chunks
IC = I // P                  # 16 I-chunks
HN = H // 512                # 2 output column chunks of 512
WG = 4                       # w13 weight-DMA grouping (I-chunks per DMA)

_program_cache: dict = {}
_last_in_maps: list | None = None


def _token_chunks(c):
    """Split c columns into chunks <=512 (PSUM bank limit), roughly even."""
    n = max(1, math.ceil(c / 512))
    base = c // n
    rem = c - base * n
    out = []
    start = 0
    for j in range(n):
        sz = base + (1 if j < rem else 0)
        out.append((start, sz))
        start += sz
    return out


def _ffn(nc, pools, xt_sbs, ct, w13t_d, w2t_d, yb_d, wv_sb, ic=IC):
    """SwiGLU FFN over ct tokens; output written as [HN, ct//128, 128, 512]
    bf16 blocks, rows scaled by wv when given.

    xt_sb: SBUF tile [128, HC, ct] (bf16) holding x transposed.
    w13t_d: DRAM [ic, 128, 2, HC, 128] bf16 pre-tiled lhsT blocks (w1|w3).
    w2t_d: DRAM [HN, 128, ic, 512] bf16 pre-tiled rhs blocks.
    """
    wpool, w2pool, actpool, tmppool, ypool, psum = pools
    chunks = _token_chunks(ct)
    mc = -(-ct // P)

    skip_wdma = getattr(_ffn, "_skip_wdma", False)
    skip_mm = getattr(_ffn, "_skip_mm", False)

    actT = actpool.tile([P, ic, ct], BF16, name="actT", tag="actT")
    # progressive w13 DMA groups: small first groups so the first matmul's
    # weights land fast, then full-width groups for bandwidth
    if ic == 16:
        groups = [1, 1, 2, 4, 4, 4]
    else:
        groups = [min(WG, ic)] * (ic // min(WG, ic))
    g_of_i, i0_of_g = [], []
    for gi, g in enumerate(groups):
        i0_of_g.append(sum(groups[:gi]))
        g_of_i.extend([gi] * g)

    wt13_c = None
    if skip_wdma:
        wt13_c = wpool.tile([P, 1, 2, HC, P], BF16, tag="wtc", name="wt13_c", bufs=1)
        nc.sync.dma_start(
            out=wt13_c, in_=w13t_d[0:1].rearrange("g p w h c -> p g w h c"))
    wt13g = None
    for i in range(ic):
        if skip_wdma:
            wt13g = wt13_c
            wt13 = wt13g[:, 0]
        else:
            gi = g_of_i[i]
            i0, g = i0_of_g[gi], groups[gi]
            if i == i0:
                wt13g = wpool.tile(
                    [P, g, 2, HC, P], BF16, tag=f"wt{g}", name="wt13g",
                    bufs=(2 if g < WG else 3))
                nc.sync.dma_start(
                    out=wt13g,
                    in_=w13t_d[i0:i0 + g].rearrange("g p w h c -> p g w h c"))
            wt13 = wt13g[:, i - i0]
        if skip_mm:
            continue
        for ci, (c0, cn) in enumerate(chunks):
            ps1 = psum.tile([P, 512], F32, name="ps1", tag="ps")[:, :cn]
            for h in range(HC):
                nc.tensor.matmul(
                    ps1, lhsT=wt13[:, 0, h, :], rhs=xt_sbs[ci][:, h, :cn],
                    start=(h == 0), stop=(h == HC - 1),
                )
            ps3 = psum.tile([P, 512], F32, name="ps3", tag="ps")[:, :cn]
            for h in range(HC):
                nc.tensor.matmul(
                    ps3, lhsT=wt13[:, 1, h, :], rhs=xt_sbs[ci][:, h, :cn],
                    start=(h == 0), stop=(h == HC - 1),
                )
            sil = tmppool.tile([P, 512], F32, name="sil")[:, :cn]
            nc.scalar.activation(sil, ps1, ActFn.Silu)
            nc.vector.tensor_mul(actT[:, i, c0:c0 + cn], sil, ps3)

    for n in range(HN):
        if skip_wdma:
            w2a = w2pool.tile([P, ic, 512], BF16, tag="w2c", name="w2c", bufs=1)
            nc.sync.dma_start(out=w2a, in_=w2t_d[0])
        else:
            w2a = w2pool.tile([P, ic, 512], BF16, tag="w2", name="w2a")
            nc.sync.dma_start(out=w2a, in_=w2t_d[n])
        if skip_mm:
            continue
        yt = ypool.tile([P, mc, 512], BF16, name="yt", tag="yt")
        half = mc // 2 if mc >= 6 else mc   # split big tiles to start the
        for m in range(mc):                 # store early and shorten the drain
            mlen = min(P, ct - m * P)      # last m-tile may be partial
            ps = psum.tile([P, 512], F32, name="ps", tag="ps")[:mlen]
            for i in range(ic):
                nc.tensor.matmul(
                    ps, lhsT=actT[:, i, m * P:m * P + mlen], rhs=w2a[:, i, :],
                    start=(i == 0), stop=(i == ic - 1),
                )
            if wv_sb is not None:
                nc.scalar.activation(yt[:mlen, m], ps,
                                     ActFn.Copy, scale=wv_sb[:mlen, m:m + 1])
            else:
                nc.scalar.activation(yt[:mlen, m], ps, ActFn.Copy)
            if half < mc and m == half - 1:
                nc.sync.dma_start(out=yb_d[n][:, :half], in_=yt[:, :half])
        if half < mc:
            nc.sync.dma_start(out=yb_d[n][:, half:], in_=yt[:, half:])
        else:
            nc.sync.dma_start(out=yb_d[n], in_=yt)


def build_program(C, nloop=1, skip_wdma=False, skip_mm=False):
    """Build + bass-compile the SPMD program for capacity C.

    skip_wdma/skip_mm are benchmarking aids (wrong results): reuse one
    resident weight tile for every matmul / drop the matmuls+evictions.
    """
    nc = bacc.Bacc("TRN2", target_bir_lowering=False, debug=False)

    MC = -(-C // P)
    xt = nc.dram_tensor("xt", [P, HC, C], BF16, kind="ExternalInput").ap()
    wv = nc.dram_tensor("wv", [MC * P], F32, kind="ExternalInput").ap()
    w13t = nc.dram_tensor("w13t", [IC, P, 2, HC, P], BF16, kind="ExternalInput").ap()
    w2t = nc.dram_tensor("w2t", [HN, P, IC, 512], BF16, kind="ExternalInput").ap()
    xst = nc.dram_tensor("xst", [P, HC, CS], BF16, kind="ExternalInput").ap()
    sw13t = nc.dram_tensor("sw13t", [SIC, P, 2, HC, P], BF16, kind="ExternalInput").ap()
    sw2t = nc.dram_tensor("sw2t", [HN, P, SIC, 512], BF16, kind="ExternalInput").ap()
    yb = nc.dram_tensor("yb", [HN, P, MC, 512], BF16, kind="ExternalOutput").ap()
    ysb = nc.dram_tensor("ysb", [HN, P, CS // P, 512], BF16, kind="ExternalOutput").ap()

    with tile.TileContext(nc) as tc:
        with ExitStack() as ctx:
            const = ctx.enter_context(tc.tile_pool(name="const", bufs=1))
            wpool = ctx.enter_context(tc.tile_pool(name="wpool", bufs=3))
            w2pool = ctx.enter_context(tc.tile_pool(name="w2pool", bufs=2))
            actpool = ctx.enter_context(tc.tile_pool(name="actpool", bufs=1))
            tmppool = ctx.enter_context(tc.tile_pool(name="tmppool", bufs=3))
            ypool = ctx.enter_context(tc.tile_pool(name="ypool", bufs=2))
            psum = ctx.enter_context(tc.tile_pool(name="psum", bufs=6, space="PSUM"))
            pools = (wpool, w2pool, actpool, tmppool, ypool, psum)

            # xt is on the first matmul's critical path: keep it alone on the
            # SP ring ahead of the weight stream; wv rides the ACT ring; xst
            # is loaded inside the body AFTER the routed weight stream so it
            # never delays the first matmul's weights.
            xt_sbs = []
            for j, (c0, cn) in enumerate(_token_chunks(C)):
                xt_c = const.tile([P, HC, cn], BF16, name=f"xt_c{c0}")
                # only chunk 0 gates the first matmul: keep it on the SP ring
                # ahead of the weight stream, later chunks on the ACT ring so
                # the first weight group isn't queued behind them
                eng = nc.sync if j == 0 else nc.scalar
                eng.dma_start(out=xt_c, in_=xt[:, :, c0:c0 + cn])
                xt_sbs.append(xt_c)
            wv_sb = const.tile([P, MC], F32)
            nc.scalar.dma_start(out=wv_sb, in_=wv.rearrange("(m p) -> p m", p=P))
            xstpool = ctx.enter_context(tc.tile_pool(name="xstpool", bufs=1))

            def body():
                _ffn._skip_wdma = skip_wdma
                _ffn._skip_mm = skip_mm
                _ffn(nc, pools, xt_sbs, C, w13t, w2t, yb, wv_sb)
                xst_sbs = []
                for (c0, cn) in _token_chunks(CS):
                    xst_c = xstpool.tile([P, HC, cn], BF16,
                                         name="xst_c", tag=f"xst{c0}")
                    nc.sync.dma_start(out=xst_c, in_=xst[:, :, c0:c0 + cn])
                    xst_sbs.append(xst_c)
                _ffn(nc, pools, xst_sbs, CS, sw13t, sw2t, ysb, None, ic=SIC)
                _ffn._skip_wdma = False
                _ffn._skip_mm = False

            if nloop == 1:
                body()
            else:
                with tc.For_i(0, nloop, 1):
                    body()
    nc.compile()
    return nc


def _route(xf, gate_w):
    """Replicate the reference routing in numpy fp32."""
    logits = xf @ gate_w.T                      # [T, E]
    m = logits.max(axis=1, keepdims=True)
    p = np.exp(logits - m)
    scores = p / p.sum(axis=1, keepdims=True)
    order = np.argsort(-scores, axis=1, kind="stable")[:, :TOPK]   # [T, 2]
    w_top = np.take_along_axis(scores, order, axis=1)
    w_top = w_top / (w_top.sum(axis=1, keepdims=True) + np.float32(1e-20))
    return order.astype(np.int64), w_top.astype(np.float32)


def _pretile_lhs13(w1, w3, ic=IC):
    # 2 x [H, i] -> [ic, 128, 2, HC, 128] bf16
    out = np.empty((ic, P, 2, HC, P), dtype=BF)
    out[:, :, 0] = w1.reshape(HC, P, ic, P).transpose(2, 1, 0, 3).astype(BF)
    out[:, :, 1] = w3.reshape(HC, P, ic, P).transpose(2, 1, 0, 3).astype(BF)
    return out


def _pretile_rhs(w, ic=IC):          # [i, H] -> [HN, 128, ic, 512] bf16
    return np.ascontiguousarray(
        w.reshape(ic, P, HN, 512).transpose(2, 1, 0, 3)).astype(BF)


def _unblock(yb, ct=None):    # [HN, 128, mc, 512] -> [mc*128, H] fp32
    mc = yb.shape[2]
    return yb.astype(np.float32).transpose(2, 1, 0, 3).reshape(mc * P, H)


def _pack_xt(xT, ct):         # [H, ct] -> [128, HC, ct] (partition-major)
    return np.ascontiguousarray(
        xT.reshape(HC, P, ct).transpose(1, 0, 2)).astype(BF)


def kernel(x, gate_w, w1, w2, w3, sw1, sw2, sw3):
    x = np.asarray(x, dtype=np.float32)
    xf = x.reshape(T, H)
    order, w_top = _route(xf, np.asarray(gate_w, dtype=np.float32))

    idxs, wts = [], []
    for e in range(E):
        m0 = order[:, 0] == e
        m1 = order[:, 1] == e
        idx = np.concatenate([np.nonzero(m0)[0], np.nonzero(m1)[0]])
        wt = np.concatenate([w_top[m0, 0], w_top[m1, 1]])
        idxs.append(idx)
        wts.append(wt.astype(np.float32))

    maxn = max(len(i) for i in idxs)
    C = max(P, ((maxn + 63) // 64) * 64)   # capacity at 64-granularity

    if C not in _program_cache:
        _program_cache[C] = build_program(C)
    nc = _program_cache[C]

    sw1_ = np.asarray(sw1, dtype=np.float32)[0]
    sw2_ = np.asarray(sw2, dtype=np.float32)[0]
    sw3_ = np.asarray(sw3, dtype=np.float32)[0]
    isz = I // SI
    sw13ts = [_pretile_lhs13(sw1_[:, s * isz:(s + 1) * isz],
                             sw3_[:, s * isz:(s + 1) * isz], ic=SIC)
              for s in range(SI)]
    sw2ts = [_pretile_rhs(sw2_[s * isz:(s + 1) * isz, :], ic=SIC)
             for s in range(SI)]
    xsts = [_pack_xt(np.ascontiguousarray(xf[g * CS:(g + 1) * CS].T), CS)
            for g in range(TG)]

    w1 = np.asarray(w1, dtype=np.float32)
    w2 = np.asarray(w2, dtype=np.float32)
    w3 = np.asarray(w3, dtype=np.float32)

    in_maps = []
    for c in range(NCORES):
        idx, wt = idxs[c], wts[c]
        n = len(idx)
        xte = np.zeros((H, C), dtype=np.float32)
        xte[:, :n] = xf[idx].T
        wve = np.zeros((-(-C // P) * P,), dtype=np.float32)
        wve[:n] = wt
        in_maps.append({
            "xt": _pack_xt(xte, C),
            "wv": wve,
            "w13t": _pretile_lhs13(w1[c], w3[c]),
            "w2t": _pretile_rhs(w2[c]),
            "xst": xsts[c // SI],
            "sw13t": sw13ts[c % SI],
            "sw2t": sw2ts[c % SI],
        })

    global _last_in_maps
    _last_in_maps = in_maps
    res = run_bass_kernel_spmd(nc, in_maps, core_ids=list(range(NCORES)))

    out = np.zeros((T, H), dtype=np.float32)
    for c in range(NCORES):
        g = c // SI
        out[g * CS:(g + 1) * CS] += _unblock(res.results[c]["ysb"], CS)
    for c in range(NCORES):
        idx = idxs[c]
        if len(idx):
            out[idx] += _unblock(res.results[c]["yb"], C)[:len(idx)]
    return out.reshape(B, S, H)



# revision 19
# speedup vs baseline: 1.0375x; 1.0375x over previous
"""Trainium2 Bass kernel for ChronosMOEFeedForward (8-expert top-2 MoE + shared expert).

All matmul data is bf16 (fp32 PSUM accumulation). Weight/activation layouts are
pre-tiled host-side so every DMA is a contiguous per-partition block.

Strategy (expert-parallel over 8 NeuronCores):
  - Host computes the (tiny) gate: softmax(x @ gate_w.T), top-2, normalized
    combine weights; dispatches each token to its 2 experts.
  - Core e runs a SwiGLU FFN over the tokens routed to expert e (gathered,
    transposed, padded to capacity C). Each core also processes a 1024-token
    x I/4 slice of the shared expert.
  - mm2 keeps H on the partition dim (w2 stationary, act streamed), so the
    token count enters only as a free dim: no 128-granularity padding and the
    output comes back transposed [H, C]. The top-2 combine weights are applied
    host-side during the scatter-add.
  - DMA queues: all loads ride the Sync/SP ring in the order the compute
    needs them (loads have no data deps, so the FIFO never head-of-line
    blocks); all output stores ride the Scalar/ACT ring.

Fixed problem shapes: x [2,1024,1024], E=8 experts, H=1024, I=2048, top-2,
one shared expert. The device program is compiled per capacity C (multiple
of 8 covering the max per-expert token count) and cached in-process.
"""
import math
from contextlib import ExitStack

import numpy as np
import ml_dtypes

import concourse.bass as bass
import concourse.tile as tile
from concourse import bacc, mybir
from concourse.bass_utils import run_bass_kernel_spmd

F32 = mybir.dt.float32
BF16 = mybir.dt.bfloat16
ActFn = mybir.ActivationFunctionType
BF = ml_dtypes.bfloat16

P = 128
B, S, H = 2, 1024, 1024
T = B * S                    # 2048 tokens
E, TOPK, I = 8, 2, 2048
NCORES = 8
TG = 2                       # shared expert: token groups
SI = 4                       # shared expert: I-dimension split
CS = T // TG                 # shared-expert tokens per core (1024)
SIC = I // P // SI           # shared-expert I-chunks per core (4)
HC = H // P                  # 8 H-chunks
IC = I // P                  # 16 I-chunks
WG = 4                       # w13 weight-DMA grouping (I-chunks per DMA)

_program_cache: dict = {}
_last_in_maps: list | None = None


def _token_chunks(c):
    """Split c columns into chunks <=512 (PSUM bank limit), roughly even."""
    n = max(1, math.ceil(c / 512))
    base = c // n
    rem = c - base * n
    out = []
    start = 0
    for j in range(n):
        sz = base + (1 if j < rem else 0)
        out.append((start, sz))
        start += sz
    return out


def _w13_groups(ic):
    """Progressive w13 DMA groups: small first groups so the first matmul's
    weights land fast, then full-width groups for bandwidth."""
    if ic == 16:
        return [1, 1, 2, 4, 4, 4]
    return [min(WG, ic)] * (ic // min(WG, ic))


def _load_w13(nc, wpool, w13t_d, ic, eng, skip=0):
    """Issue the w13 group DMAs on `eng`; return per-i-chunk weight views
    for i-chunks skip..ic-1 (the first `skip` chunks live in persistent
    prefetch tiles owned by the caller)."""
    groups = _w13_groups(ic)
    views = []
    i0 = sum(groups[:skip])
    for g in groups[skip:]:
        wt13g = wpool.tile(
            [P, g, 2, HC, P], BF16, tag=f"wt{g}", name="wt13g",
            bufs=(2 if g < WG else 3))
        eng.dma_start(
            out=wt13g,
            in_=w13t_d[i0:i0 + g].rearrange("g p w h c -> p g w h c"))
        views.extend(wt13g[:, k] for k in range(g))
        i0 += g
    return views


def _load_w2(nc, w2pool, w2t_d, ic, eng, tag, bufs):
    """Issue the 8 per-hc w2 stationary-block DMAs on `eng`."""
    tiles = []
    for hc in range(HC):
        w2a = w2pool.tile([P, ic, P], BF16, tag=tag, name="w2a", bufs=bufs)
        eng.dma_start(out=w2a, in_=w2t_d[hc])
        tiles.append(w2a)
    return tiles


def _mm13(nc, pools, xt_sbs, ct, w13v, ic):
    """h1/h3 matmuls + silu*mul; returns actT [128, ic, ct] bf16."""
    wpool, w2pool, actpool, tmppool, ypool, psum = pools
    chunks = _token_chunks(ct)
    actT = actpool.tile([P, ic, ct], BF16, name="actT", tag="actT")
    for i in range(ic):
        wt13 = w13v[i]
        for ci, (c0, cn) in enumerate(chunks):
            ps1 = psum.tile([P, 512], F32, name="ps1", tag="ps")[:, :cn]
            for h in range(HC):
                nc.tensor.matmul(
                    ps1, lhsT=wt13[:, 0, h, :], rhs=xt_sbs[ci][:, h, :cn],
                    start=(h == 0), stop=(h == HC - 1),
                )
            ps3 = psum.tile([P, 512], F32, name="ps3", tag="ps")[:, :cn]
            for h in range(HC):
                nc.tensor.matmul(
                    ps3, lhsT=wt13[:, 1, h, :], rhs=xt_sbs[ci][:, h, :cn],
                    start=(h == 0), stop=(h == HC - 1),
                )
            sil = tmppool.tile([P, 512], F32, name="sil")[:, :cn]
            nc.scalar.activation(sil, ps1, ActFn.Silu)
            nc.vector.tensor_mul(actT[:, i, c0:c0 + cn], sil, ps3)
    return actT


def _mm2(nc, pools, actT, ct, w2tiles, yb_d, ic, out_eng, alt_evict=False):
    """w2 stationary [I-part, H-cols], actT streamed (tokens free dim);
    output transposed [HC, 128, ct] bf16, stores issued on `out_eng`.

    alt_evict alternates PSUM evictions between Vector and Scalar so neither
    engine saturates during a short mm2 phase (PSUM-bank recycling would
    otherwise gate the matmuls)."""
    wpool, w2pool, actpool, tmppool, ypool, psum = pools
    chunks = _token_chunks(ct)
    for hc in range(HC):
        w2a = w2tiles[hc]
        pss = [psum.tile([P, 512], F32, name="ps2", tag="ps")[:, :cn]
               for (c0, cn) in chunks]
        for i in range(ic):
            for ci, (c0, cn) in enumerate(chunks):
                nc.tensor.matmul(
                    pss[ci], lhsT=w2a[:, i, :], rhs=actT[:, i, c0:c0 + cn],
                    start=(i == 0), stop=(i == ic - 1),
                )
        for ci, (c0, cn) in enumerate(chunks):
            yt = ypool.tile([P, 512], BF16, name="yt", tag="yt")[:, :cn]
            if alt_evict and (hc * len(chunks) + ci) % 2 == 1:
                nc.scalar.activation(yt, pss[ci], ActFn.Copy)
            else:
                nc.vector.tensor_copy(yt, pss[ci])
            out_eng.dma_start(out=yb_d[hc][:, c0:c0 + cn], in_=yt)


def build_program(C, nloop=1, staggered=False, unroll=8):
    """Build + bass-compile the SPMD program for capacity C.

    For nloop>1 the body is unrolled `unroll`x inside the hardware loop so
    the For_i all-engine reset barrier (and the HAM re-throttle its PE idle
    triggers) is paid once per `unroll` iterations; at the unrolled
    junctions the weight stream prefetches across and the PE never idles.
    Any nloop is handled: remainder iterations run after the loop.
    """
    nc = bacc.Bacc("TRN2", target_bir_lowering=False, debug=False)

    xt = nc.dram_tensor("xt", [P, HC, C], BF16, kind="ExternalInput").ap()
    w13t = nc.dram_tensor("w13t", [IC, P, 2, HC, P], BF16, kind="ExternalInput").ap()
    w2t = nc.dram_tensor("w2t", [HC, P, IC, P], BF16, kind="ExternalInput").ap()
    xst = nc.dram_tensor("xst", [P, HC, CS], BF16, kind="ExternalInput").ap()
    sw13t = nc.dram_tensor("sw13t", [SIC, P, 2, HC, P], BF16, kind="ExternalInput").ap()
    sw2t = nc.dram_tensor("sw2t", [HC, P, SIC, P], BF16, kind="ExternalInput").ap()
    yb = nc.dram_tensor("yb", [HC, P, C], BF16, kind="ExternalOutput").ap()
    ysb = nc.dram_tensor("ysb", [HC, P, CS], BF16, kind="ExternalOutput").ap()

    with tile.TileContext(nc) as tc:
        with ExitStack() as ctx:
            const = ctx.enter_context(tc.tile_pool(name="const", bufs=1))
            wpool = ctx.enter_context(tc.tile_pool(name="wpool", bufs=3))
            w2pool = ctx.enter_context(tc.tile_pool(name="w2pool", bufs=3))
            actpool = ctx.enter_context(tc.tile_pool(name="actpool", bufs=2))
            tmppool = ctx.enter_context(tc.tile_pool(name="tmppool", bufs=3))
            ypool = ctx.enter_context(tc.tile_pool(name="ypool", bufs=6))
            psum = ctx.enter_context(tc.tile_pool(name="psum", bufs=7, space="PSUM"))
            pools = (wpool, w2pool, actpool, tmppool, ypool, psum)

            # xt is on the first matmul's critical path: chunk 0 leads the SP
            # ring ahead of the weight stream; later chunks ride the ACT ring
            # so the first weight group isn't queued behind them.
            xt_sbs = []
            for j, (c0, cn) in enumerate(_token_chunks(C)):
                xt_c = const.tile([P, HC, cn], BF16, name=f"xt_c{c0}")
                eng = nc.sync if j == 0 else nc.scalar
                eng.dma_start(out=xt_c, in_=xt[:, :, c0:c0 + cn])
                xt_sbs.append(xt_c)
            xstpool = ctx.enter_context(tc.tile_pool(name="xstpool", bufs=2))

            # Persistent tiles for the first two w13 i-chunks: the body
            # RE-loads them late each iteration (same SBUF address), so after
            # the For_i reset barrier the first matmuls never wait on a DMA.
            npre = 2
            pre = [const.tile([P, 1, 2, HC, P], BF16, name=f"w13pre{k}")
                   for k in range(npre)]
            for k in range(npre):
                nc.sync.dma_start(
                    out=pre[k],
                    in_=w13t[k:k + 1].rearrange("g p w h c -> p g w h c"))

            def body():
                # ALL loads ride the SP (sync) ring, emitted up front in
                # need-order (loads have no data deps, so the FIFO streams
                # continuously and the NEXT body's first weight group
                # prefetches right after). Scalar's queue starts with compute
                # (silu) so post-barrier PSUM recycling isn't delayed by DMA
                # issue work.
                w13v_r = [pre[k][:, 0] for k in range(npre)] + \
                    _load_w13(nc, wpool, w13t, IC, nc.sync, skip=npre)
                w2_r = _load_w2(nc, w2pool, w2t, IC, nc.sync, "w2r", 6)
                actT_r = _mm13(nc, pools, xt_sbs, C, w13v_r, IC)
                xst_sbs = []
                for (c0, cn) in _token_chunks(CS):
                    xst_c = xstpool.tile([P, HC, cn], BF16,
                                         name="xst_c", tag=f"xst{c0}")
                    nc.sync.dma_start(out=xst_c, in_=xst[:, :, c0:c0 + cn])
                    xst_sbs.append(xst_c)
                w13v_s = _load_w13(nc, wpool, sw13t, SIC, nc.sync)
                w2_s = _load_w2(nc, w2pool, sw2t, SIC, nc.sync, "w2s", 8)
                # shared phases mid-body: the relaxed 30us routed mm2 forms
                # the body tail, so trailing PSUM evictions never gate the
                # next body's start. Shared-mm2 stores ride Sync (they clear
                # mid-body, before the next body's weight prefetch), routed
                # stores ride Scalar.
                actT_s = _mm13(nc, pools, xst_sbs, CS, w13v_s, SIC)
                _mm2(nc, pools, actT_s, CS, w2_s, ysb, SIC,
                     out_eng=nc.sync, alt_evict=True)
                # refresh the persistent first w13 groups for the NEXT body
                # (same data; keeps the steady-state DMA traffic honest and
                # the post-barrier critical path DMA-free)
                for k in range(npre):
                    nc.sync.dma_start(
                        out=pre[k],
                        in_=w13t[k:k + 1].rearrange("g p w h c -> p g w h c"))
                _mm2(nc, pools, actT_r, C, w2_r, yb, IC, out_eng=nc.scalar)

            if nloop == 1:
                body()
            else:
                main = (nloop // unroll) * unroll
                if main:
                    with tc.For_i(0, main, unroll, staggered_reset=staggered):
                        for _ in range(unroll):
                            body()
                for _ in range(nloop - main):
                    body()
    nc.compile()
    return nc


def _route(xf, gate_w):
    """Replicate the reference routing in numpy fp32."""
    logits = xf @ gate_w.T                      # [T, E]
    m = logits.max(axis=1, keepdims=True)
    p = np.exp(logits - m)
    scores = p / p.sum(axis=1, keepdims=True)
    order = np.argsort(-scores, axis=1, kind="stable")[:, :TOPK]   # [T, 2]
    w_top = np.take_along_axis(scores, order, axis=1)
    w_top = w_top / (w_top.sum(axis=1, keepdims=True) + np.float32(1e-20))
    return order.astype(np.int64), w_top.astype(np.float32)


def _pretile_lhs13(w1, w3, ic=IC):
    # 2 x [H, i] -> [ic, 128, 2, HC, 128] bf16
    out = np.empty((ic, P, 2, HC, P), dtype=BF)
    out[:, :, 0] = w1.reshape(HC, P, ic, P).transpose(2, 1, 0, 3).astype(BF)
    out[:, :, 1] = w3.reshape(HC, P, ic, P).transpose(2, 1, 0, 3).astype(BF)
    return out


def _pretile_w2T(w, ic=IC):    # [i, H] -> [HC, 128, ic, 128] bf16
    return np.ascontiguousarray(
        w.reshape(ic, P, HC, P).transpose(2, 1, 0, 3)).astype(BF)


def _unblockT(yb):    # [HC, 128, ct] -> [ct, H] fp32
    ct = yb.shape[2]
    return yb.astype(np.float32).transpose(2, 0, 1).reshape(ct, H)


def _pack_xt(xT, ct):         # [H, ct] -> [128, HC, ct] (partition-major)
    return np.ascontiguousarray(
        xT.reshape(HC, P, ct).transpose(1, 0, 2)).astype(BF)


def kernel(x, gate_w, w1, w2, w3, sw1, sw2, sw3):
    x = np.asarray(x, dtype=np.float32)
    xf = x.reshape(T, H)
    order, w_top = _route(xf, np.asarray(gate_w, dtype=np.float32))

    idxs, wts = [], []
    for e in range(E):
        m0 = order[:, 0] == e
        m1 = order[:, 1] == e
        idx = np.concatenate([np.nonzero(m0)[0], np.nonzero(m1)[0]])
        wt = np.concatenate([w_top[m0, 0], w_top[m1, 1]])
        idxs.append(idx)
        wts.append(wt.astype(np.float32))

    maxn = max(len(i) for i in idxs)
    C = max(P, ((maxn + 7) // 8) * 8)      # capacity at 8-granularity

    if C not in _program_cache:
        _program_cache[C] = build_program(C)
    nc = _program_cache[C]

    sw1_ = np.asarray(sw1, dtype=np.float32)[0]
    sw2_ = np.asarray(sw2, dtype=np.float32)[0]
    sw3_ = np.asarray(sw3, dtype=np.float32)[0]
    isz = I // SI
    sw13ts = [_pretile_lhs13(sw1_[:, s * isz:(s + 1) * isz],
                             sw3_[:, s * isz:(s + 1) * isz], ic=SIC)
              for s in range(SI)]
    sw2ts = [_pretile_w2T(sw2_[s * isz:(s + 1) * isz, :], ic=SIC)
             for s in range(SI)]
    xsts = [_pack_xt(np.ascontiguousarray(xf[g * CS:(g + 1) * CS].T), CS)
            for g in range(TG)]

    w1 = np.asarray(w1, dtype=np.float32)
    w2 = np.asarray(w2, dtype=np.float32)
    w3 = np.asarray(w3, dtype=np.float32)

    in_maps = []
    for c in range(NCORES):
        idx = idxs[c]
        n = len(idx)
        xte = np.zeros((H, C), dtype=np.float32)
        xte[:, :n] = xf[idx].T
        in_maps.append({
            "xt": _pack_xt(xte, C),
            "w13t": _pretile_lhs13(w1[c], w3[c]),
            "w2t": _pretile_w2T(w2[c]),
            "xst": xsts[c // SI],
            "sw13t": sw13ts[c % SI],
            "sw2t": sw2ts[c % SI],
        })

    global _last_in_maps
    _last_in_maps = in_maps
    res = run_bass_kernel_spmd(nc, in_maps, core_ids=list(range(NCORES)))

    out = np.zeros((T, H), dtype=np.float32)
    for c in range(NCORES):
        g = c // SI
        out[g * CS:(g + 1) * CS] += _unblockT(res.results[c]["ysb"])
    for c in range(NCORES):
        idx = idxs[c]
        if len(idx):
            out[idx] += wts[c][:, None] * _unblockT(res.results[c]["yb"])[:len(idx)]
    return out.reshape(B, S, H)
